# revision 10
# baseline (speedup 1.0000x reference)
"""Trainium2 Bass kernel for nn_Decoder (GNN message passing decoder).

Host sorts graphs by num_nodes into uniform blocks (no masks on device),
shards round-robin over 8 cores (one SPMD program). Device uses a
feature-major layout. Node/graph stage (glob MLP + GIN + GraphNorm) runs
fp32 matmuls by default (the network amplifies per-stage rounding ~40x);
the shallow late stage (pred/feat/edge MLPs) runs fp16 matmuls.
KERNEL_FAST=1 switches the node stage to fp32r (1 cyc/row, ~1.2e-2 err).

GIN layer 0 is folded into per-block constants; aggregation uses
linearity; GraphNorm stats come from the raw PSUM accumulator in fp32.
The edge MLP computes only the n(n+1)/2 unique pairs per graph; the host
mirrors, adds final biases, and scatters into the padded outputs.
"""

import os
import sys
from contextlib import ExitStack

import numpy as np

for _p in ("/opt/trn_rl_repo", os.path.expanduser("~/.axon_site/_ro/trn_rl_repo")):
    if os.path.isdir(_p) and _p not in sys.path:
        sys.path.append(_p)

import concourse.bacc as bacc
import concourse.bass as bass
import concourse.tile as tile
from concourse import mybir
from concourse.bass_utils import run_bass_kernel_spmd

F32 = mybir.dt.float32
F32R = mybir.dt.float32r
F16 = mybir.dt.float16
AF = mybir.ActivationFunctionType
ALU = mybir.AluOpType
AX = mybir.AxisListType

NCORES = 8
MAXN = 9
NEG = 0.01
EPS = 1e-5
PC = 512

FAST = os.environ.get("KERNEL_FAST", "0") == "1"


def _np(x):
    return np.asarray(x, dtype=np.float32)


class _Packer:
    def __init__(self, dtype):
        self.cols = 0
        self.items = []
        self.dtype = dtype

    def add(self, arr, pad_m=None):
        arr = _np(arr)
        assert arr.ndim == 2 and arr.shape[0] <= 128
        if pad_m is not None and arr.shape[1] < pad_m:
            arr = np.pad(arr, ((0, 0), (0, pad_m - arr.shape[1])))
        off = self.cols
        self.cols += arr.shape[1]
        self.items.append((off, arr))
        return (off, arr.shape[0], arr.shape[1])

    def materialize(self):
        out = np.zeros((128, max(self.cols, 2)), np.float32)
        for off, arr in self.items:
            out[: arr.shape[0], off : off + arr.shape[1]] = arr
        return out.astype(self.dtype)


def _prep_host(params, lap_table, num_nodes_arr):
    p = params
    lap = _np(lap_table)

    G_ns = []
    core_ids_by_block = []
    for n in range(1, MAXN + 1):
        ids = np.where(num_nodes_arr == n)[0]
        per_core = [ids[c::NCORES] for c in range(NCORES)]
        G_n = max(len(x) for x in per_core) if len(ids) else 0
        G_n = (G_n + 1) & ~1
        G_ns.append(G_n)
        padded = []
        for c in range(NCORES):
            a = np.full(G_n, -1, np.int64)
            a[: len(per_core[c])] = per_core[c]
            padded.append(a)
        core_ids_by_block.append(padded)

    chunks = []
    for n in range(1, MAXN + 1):
        G = G_ns[n - 1]
        if G == 0:
            continue
        max_g = max(2, (PC // n) & ~1)
        s = 0
        while s < G:
            gk = min(max_g, G - s)
            chunks.append((n, s, gk))
            s += gk
    NCH = len(chunks)

    # node space: bin-pack chunks into 512-col bins (first-fit decreasing)
    order = sorted(range(NCH), key=lambda ci: -chunks[ci][0] * chunks[ci][2])
    bins = []
    node_off = [0] * NCH
    for ci in order:
        cols = chunks[ci][0] * chunks[ci][2]
        for bi, bcur in enumerate(bins):
            if bcur[0] + cols <= PC:
                node_off[ci] = bi * PC + bcur[0]
                bcur[0] += cols
                break
        else:
            node_off[ci] = len(bins) * PC
            bins.append([cols])
    N_ndp = len(bins) * PC

    pair_off = []
    N_pr = 0
    for (n, s, gk) in chunks:
        pair_off.append(N_pr)
        N_pr += (n * (n + 1) // 2) * gk
    N_prp = ((N_pr + PC - 1) // PC) * PC

    goff = []
    G_c = 0
    for (n, s, gk) in chunks:
        goff.append(G_c)
        G_c += gk

    Wf = _Packer(np.float32)
    Wh = _Packer(np.float16)
    Bp = _Packer(np.float32)

    def lin(t):
        return _np(t[0]), _np(t[1])

    wn1, bn1 = lin(p["num_net"][0]); wn2, bn2 = lin(p["num_net"][1]); wn3, bn3 = lin(p["num_net"][2])
    wg1, bg1 = lin(p["glob"][0]); wg2, bg2 = lin(p["glob"][1]); wg3, bg3 = lin(p["glob"][2])
    gins = [(lin(p["gin"][l]["A"]), lin(p["gin"][l]["B"])) for l in range(3)]
    gamma = [_np(p["gn"][l]["gamma"]) for l in range(2)]
    beta = [_np(p["gn"][l]["beta"]) for l in range(2)]
    alpha = [_np(p["gn"][l]["alpha"]) for l in range(2)]
    wf1, bf1 = lin(p["feat"][0]); wf2, bf2 = lin(p["feat"][1]); wf3, bf3 = lin(p["feat"][2])
    we1, be1 = lin(p["edge"][0]); we2, be2 = lin(p["edge"][1]); we3, be3 = lin(p["edge"][2])

    wref = {}
    wref["wg1"] = Wf.add(wg1, pad_m=128)
    wref["wg2"] = Wf.add(wg2, pad_m=128)
    wref["wg3"] = Wf.add(wg3, pad_m=128)
    wa0, ba0 = gins[0][0]
    for n in range(1, MAXN + 1):
        wref[f"wa0_{n}"] = Wf.add(wa0[:64, :] * (1.0 + n))
    wref["wb0"] = Wf.add(gins[0][1][0])
    wref["wa1"] = Wf.add(gins[1][0][0]); wref["wb1"] = Wf.add(gins[1][1][0])
    wref["wa2"] = Wf.add(gins[2][0][0]); wref["wb2"] = Wf.add(gins[2][1][0])

    href = {}
    href["wn1"] = Wh.add(wn1); href["wn2"] = Wh.add(wn2); href["wn3"] = Wh.add(wn3)
    href["wf1"] = Wh.add(wf1); href["wf2"] = Wh.add(wf2); href["wf3"] = Wh.add(wf3)
    href["we1h"] = Wh.add(we1 * 0.5)
    href["we2"] = Wh.add(we2); href["we3"] = Wh.add(we3)

    bref = {}
    bref["bn1"] = Bp.add(bn1[:, None]); bref["bn2"] = Bp.add(bn2[:, None])
    bref["bg1"] = Bp.add(bg1[:, None]); bref["bg2"] = Bp.add(bg2[:, None])
    bref["bg3"] = Bp.add(bg3[:, None])
    bref["ba1"] = Bp.add(gins[1][0][1][:, None])
    bref["ba2"] = Bp.add(gins[2][0][1][:, None])
    bref["bb2"] = Bp.add(gins[2][1][1][:, None])
    bref["bf1"] = Bp.add(bf1[:, None]); bref["bf2"] = Bp.add(bf2[:, None])
    bref["be1h"] = Bp.add(0.5 * be1[:, None]); bref["be2"] = Bp.add(be2[:, None])
    bref["eps"] = Bp.add(np.full((128, 1), EPS, np.float32))
    for l in range(2):
        bb = gins[l][1][1]
        bref[f"gn{l}_ao"] = Bp.add(
            np.stack([alpha[l] / n for n in range(1, MAXN + 1)], 1))
        bref[f"gn{l}_ab"] = Bp.add(((alpha[l] - 1.0) * bb)[:, None])
        bref[f"gn{l}_ng"] = Bp.add((-gamma[l])[:, None])
        bref[f"gn{l}_bt"] = Bp.add(beta[l][:, None])
    for n in range(1, MAXN + 1):
        lapn = lap[n - 1]
        lsum = lapn.sum(0)
        C = (lapn[:n] + lsum[None, :]) @ wa0[64:73, :] + ba0[None, :]
        bref[f"ct_{n}"] = Bp.add(C.T)

    return dict(
        G_ns=G_ns, core_ids_by_block=core_ids_by_block, chunks=chunks,
        node_off=node_off, pair_off=pair_off, goff=goff,
        N_ndp=N_ndp, N_pr=N_pr, N_prp=N_prp, G_c=G_c,
        WF=Wf.materialize(), WH=Wh.materialize(), BP=Bp.materialize(),
        wref=wref, href=href, bref=bref,
        bn3=bn3, bf3=bf3, be3=be3,
    )


def _emit(tc, nc, host, gv_d, pn_d, nf_d, ea_d, wf_d, wh_d, bp_d):
    chunks = host["chunks"]
    node_off = host["node_off"]; pair_off = host["pair_off"]; goff = host["goff"]
    N_ndp = host["N_ndp"]; N_pr = host["N_pr"]; N_prp = host["N_prp"]
    G_c = host["G_c"]
    wref = host["wref"]; href = host["href"]; bref = host["bref"]
    ND = F32R if FAST else F32
    lp = nc.allow_low_precision

    ctx = ExitStack()
    with ctx:
        const = ctx.enter_context(tc.tile_pool(name="const", bufs=1))
        big = ctx.enter_context(tc.tile_pool(name="big", bufs=1))
        gsp = ctx.enter_context(tc.tile_pool(name="gsp", bufs=1))
        tmp = ctx.enter_context(tc.tile_pool(name="tmp", bufs=2))
        ptmp = ctx.enter_context(tc.tile_pool(name="ptmp", bufs=2))
        psA = ctx.enter_context(tc.tile_pool(name="psA", bufs=2, space="PSUM"))
        psB = ctx.enter_context(tc.tile_pool(name="psB", bufs=2, space="PSUM"))

        # ---------- constants ----------
        wfc = const.tile([128, host["WF"].shape[1]], ND)
        wcols = host["WF"].shape[1]
        step = max(64, ((wcols + 7) // 8 + 1) & ~1)
        for s in range(0, wcols, step):
            e = min(wcols, s + step)
            src = wf_d.bitcast(F32R) if FAST else wf_d
            nc.sync.dma_start(out=wfc[:, s:e], in_=src[:, s:e])
        whc = const.tile([128, host["WH"].shape[1]], F16)
        hcols = host["WH"].shape[1]
        step = max(64, ((hcols + 3) // 4 + 1) & ~1)
        for s in range(0, hcols, step):
            e = min(hcols, s + step)
            nc.sync.dma_start(out=whc[:, s:e], in_=wh_d[:, s:e])
        bpc = const.tile([128, host["BP"].shape[1]], F32)
        nc.sync.dma_start(out=bpc, in_=bp_d)

        def w(name):
            off, k, m = wref[name]
            return wfc[:k, off : off + m]

        def hw(name):
            off, k, m = href[name]
            return whc[:k, off : off + m]

        def b(name, col=0, ncol=1):
            off, k, m = bref[name]
            return bpc[:k, off + col : off + col + ncol]

        gv = const.tile([128, G_c], ND)
        gstep = ((G_c + 3) // 4 + 1) & ~1
        for s in range(0, G_c, gstep):
            e = min(G_c, s + gstep)
            src = gv_d.bitcast(F32R) if FAST else gv_d
            nc.sync.dma_start(out=gv[:, s:e], in_=src[:, s:e])
        gv16 = const.tile([128, G_c], F16)
        nc.vector.tensor_copy(out=gv16[:], in_=gv[:].bitcast(F32))

        # graph-space mm + evict helper
        def g_mm_evict(dst, lhsT, rhs, func, bias, alpha=NEG, scale=1.0):
            mp = lhsT.shape[-1]
            dp = dst.shape[0]
            s = 0
            while s < G_c:
                k = min(PC, G_c - s)
                ps = psB.tile([128, PC], F32, tag="B")
                nc.tensor.matmul(ps[:mp, :k], lhsT, rhs[:, s : s + k],
                                 start=True, stop=True)
                nc.scalar.activation(out=dst[:, s : s + k], in_=ps[:dp, :k],
                                     func=func, bias=bias, scale=scale,
                                     alpha=alpha)
                s += k

        # ---------- pred_num MLP (fp16) ----------
        s1 = gsp.tile([128, G_c], F16)
        g_mm_evict(s1, hw("wn1"), gv16, AF.Prelu, b("bn1"), alpha=0.0)
        s2 = gsp.tile([128, G_c], F16)
        g_mm_evict(s2, hw("wn2"), s1, AF.Prelu, b("bn2"), alpha=0.0)
        pnt = gsp.tile([1, G_c], F32)
        g_mm_evict(pnt, hw("wn3"), s2, AF.Copy, 0.0)
        nc.sync.dma_start(out=pn_d, in_=pnt[:])

        # ---------- glob MLP (node dtype) ----------
        g1 = gsp.tile([64, G_c], ND)
        g_mm_evict(g1, w("wg1"), gv, AF.Prelu, b("bg1"))
        g2 = gsp.tile([64, G_c], ND)
        g_mm_evict(g2, w("wg2"), g1, AF.Prelu, b("bg2"))
        g64 = gsp.tile([64, G_c], ND)
        g_mm_evict(g64, w("wg3"), g2, AF.Identity, b("bg3"))

        # ---------- GIN layer 0 ----------
        # per chunk: q = g64_chunk @ wa0_n  (psum, graph space)
        #            z0 = q bcast-over-nodes + CT[n] bcast-over-graphs (stt)
        #            a0 = prelu(z0)  (ACT)
        a_cur = big.tile([128, N_ndp], ND, tag="aA")
        for ci, (n, gs, gk) in enumerate(chunks):
            no, go = node_off[ci], goff[ci]
            cols = n * gk
            psq = psA.tile([128, 2 * PC], F32, tag="A")
            nc.tensor.matmul(psq[:, :gk], w(f"wa0_{n}"), gv64_slice(g64, go, gk),
                             start=True, stop=True)
            z0 = tmp.tile([128, PC], F32, tag="w1")
            qb = psq[:, :gk].unsqueeze(1).broadcast_to([128, n, gk])
            ctb = b(f"ct_{n}", 0, n).unsqueeze(2).broadcast_to([128, n, gk])
            nc.vector.scalar_tensor_tensor(
                out=z0[:, :cols].rearrange("p (n g) -> p n g", n=n),
                in0=qb, scalar=1.0, in1=ctb, op0=ALU.mult, op1=ALU.add)
            with lp(reason="nd rounding"):
                nc.scalar.activation(out=a_cur[:, no : no + cols],
                                     in_=z0[:, :cols], func=AF.Prelu,
                                     scale=1.0, alpha=NEG)

        # ---------- GIN layers ----------
        ta = gsp.tile([128, G_c], F32)
        t1 = gsp.tile([128, G_c], F32)
        t2g = gsp.tile([128, G_c], F32)
        am2 = gsp.tile([128, G_c], F32)
        sdev = gsp.tile([128, G_c], F32)
        rinv = gsp.tile([128, G_c], F32)
        rscr = gsp.tile([128, G_c], F32)

        h3 = None
        NMEGA = (N_ndp + 2 * PC - 1) // (2 * PC)
        mega_chunks = [[] for _ in range(NMEGA)]
        for ci in range(len(chunks)):
            mega_chunks[node_off[ci] // (2 * PC)].append(ci)

        for l in range(3):
            wb_name = f"wb{l}"
            if l < 2:
                ctil_full = big.tile([128, N_ndp], F32, tag="ctil")
                for mi in range(NMEGA):
                    raw = psA.tile([128, 2 * PC], F32, tag="A")
                    mbase = mi * 2 * PC
                    for ci in mega_chunks[mi]:
                        n, gs, gk = chunks[ci]
                        no, cols = node_off[ci], n * gk
                        mo = no - mbase
                        nc.tensor.matmul(raw[:, mo : mo + cols], w(wb_name),
                                         a_cur[:, no : no + cols],
                                         start=True, stop=True)
                    for ci in mega_chunks[mi]:
                        n, gs, gk = chunks[ci]
                        no, cols, go = node_off[ci], n * gk, goff[ci]
                        mo = no - mbase
                        raws = raw[:, mo : mo + cols]
                        nc.vector.tensor_reduce(
                            out=t1[:, go : go + gk],
                            in_=raws.rearrange("p (n g) -> p g n", n=n),
                            axis=AX.X, op=ALU.add)
                        nc.vector.tensor_scalar(
                            out=am2[:, go : go + gk], in0=t1[:, go : go + gk],
                            scalar1=b(f"gn{l}_ao", n - 1),
                            scalar2=b(f"gn{l}_ab"),
                            op0=ALU.mult, op1=ALU.add)
                        am2b = am2[:, go : go + gk].unsqueeze(1).broadcast_to(
                            [128, n, gk])
                        nc.vector.scalar_tensor_tensor(
                            out=ctil_full[:, no : no + cols].rearrange(
                                "p (n g) -> p n g", n=n),
                            in0=am2b, scalar=1.0,
                            in1=raws.rearrange("p (n g) -> p n g", n=n),
                            op0=ALU.mult, op1=ALU.subtract)
                        sq = tmp.tile([128, PC], F32, tag="w2")
                        nc.scalar.activation(out=sq[:, :cols],
                                             in_=ctil_full[:, no : no + cols],
                                             func=AF.Square, scale=1.0)
                        nc.vector.tensor_reduce(
                            out=t2g[:, go : go + gk],
                            in_=sq[:, :cols].rearrange("p (n g) -> p g n", n=n),
                            axis=AX.X, op=ALU.add)
                for ci, (n, gs, gk) in enumerate(chunks):
                    go = goff[ci]
                    nc.scalar.activation(out=sdev[:, go : go + gk],
                                         in_=t2g[:, go : go + gk],
                                         func=AF.Sqrt, scale=1.0 / n,
                                         bias=b("eps"))
                nc.vector.reciprocal_approx_accurate(out=rinv[:], in_=sdev[:],
                                                     scratch=rscr[:])
                h_next = big.tile([128, N_ndp], ND,
                                  tag="aB" if l == 0 else "aA")
                for ci, (n, gs, gk) in enumerate(chunks):
                    no, cols, go = node_off[ci], n * gk, goff[ci]
                    d = tmp.tile([128, PC], F32, tag="w3")
                    rb = rinv[:, go : go + gk].unsqueeze(1).broadcast_to(
                        [128, n, gk])
                    nc.gpsimd.tensor_tensor(
                        out=d[:, :cols].rearrange("p (n g) -> p n g", n=n),
                        in0=ctil_full[:, no : no + cols].rearrange(
                            "p (n g) -> p n g", n=n),
                        in1=rb, op=ALU.mult)
                    with lp(reason="nd rounding"):
                        nc.scalar.activation(out=h_next[:, no : no + cols],
                                             in_=d[:, :cols], func=AF.Prelu,
                                             bias=b(f"gn{l}_bt"),
                                             scale=b(f"gn{l}_ng"), alpha=NEG)
                s_nd = big.tile([128, N_ndp], ND, tag="snd")
                for ci, (n, gs, gk) in enumerate(chunks):
                    no, cols, go = node_off[ci], n * gk, goff[ci]
                    with lp(reason="agg sum"):
                        nc.vector.tensor_reduce(
                            out=ta[:, go : go + gk],
                            in_=h_next[:, no : no + cols].bitcast(F32).rearrange(
                                "p (n g) -> p g n", n=n),
                            axis=AX.X, op=ALU.add)
                    tab = ta[:, go : go + gk].unsqueeze(1).broadcast_to(
                        [128, n, gk])
                    with lp(reason="nd rounding"):
                        nc.gpsimd.tensor_tensor(
                            out=s_nd[:, no : no + cols].rearrange(
                                "p (n g) -> p n g", n=n),
                            in0=h_next[:, no : no + cols].bitcast(F32).rearrange(
                                "p (n g) -> p n g", n=n),
                            in1=tab, op=ALU.add)
                a_out = big.tile([128, N_ndp], ND,
                                 tag="aA" if l == 0 else "aB")
                for ci, (n, gs, gk) in enumerate(chunks):
                    no, cols = node_off[ci], n * gk
                    ps = psB.tile([128, PC], F32, tag="B")
                    nc.tensor.matmul(ps[:, :cols], w(f"wa{l+1}"),
                                     s_nd[:, no : no + cols],
                                     start=True, stop=True)
                    with lp(reason="nd rounding"):
                        nc.scalar.activation(out=a_out[:, no : no + cols],
                                             in_=ps[:, :cols], func=AF.Prelu,
                                             bias=b(f"ba{l+1}"), scale=1.0,
                                             alpha=NEG)
                a_cur = a_out
            else:
                h3 = big.tile([128, N_ndp], F16, tag="h3")
                for mi in range(NMEGA):
                    raw = psA.tile([128, 2 * PC], F32, tag="A")
                    mbase = mi * 2 * PC
                    for ci in mega_chunks[mi]:
                        n, gs, gk = chunks[ci]
                        no, cols = node_off[ci], n * gk
                        mo = no - mbase
                        nc.tensor.matmul(raw[:, mo : mo + cols], w(wb_name),
                                         a_cur[:, no : no + cols],
                                         start=True, stop=True)
                    mcols = min(2 * PC, N_ndp - mbase)
                    nc.scalar.activation(out=h3[:, mbase : mbase + mcols],
                                         in_=raw[:, :mcols],
                                         func=AF.Identity, bias=b("bb2"),
                                         scale=1.0)

        # ---------- feat MLP (fp16) + col-tiled wf3 ----------
        NWIN = N_ndp // PC
        f2 = big.tile([128, N_ndp], F16, tag="f2")
        for wi in range(NWIN):
            o = wi * PC
            ps = psB.tile([128, PC], F32, tag="B")
            nc.tensor.matmul(ps[:, :PC], hw("wf1"), h3[:, o : o + PC],
                             start=True, stop=True)
            f1 = tmp.tile([128, PC], F16, tag="f1")
            nc.scalar.activation(out=f1[:], in_=ps[:, :PC], func=AF.Prelu,
                                 bias=b("bf1"), scale=1.0, alpha=NEG)
            ps2 = psB.tile([128, PC], F32, tag="B")
            nc.tensor.matmul(ps2[:, :PC], hw("wf2"), f1[:],
                             start=True, stop=True)
            nc.scalar.activation(out=f2[:, o : o + PC], in_=ps2[:, :PC],
                                 func=AF.Prelu, bias=b("bf2"), scale=1.0,
                                 alpha=NEG)
        # wf3: 4 windows per psum tile via col tiling
        for wg in range(0, NWIN, 4):
            nw = min(4, NWIN - wg)
            ps3 = psA.tile([128, 2 * PC], F32, tag="A")
            for j in range(nw):
                o = (wg + j) * PC
                nc.tensor.matmul(ps3[32 * j : 32 * j + 4, :PC], hw("wf3"),
                                 f2[:, o : o + PC], start=True, stop=True,
                                 tile_position=(0, 32 * j))
            nfs = tmp.tile([128, PC], F32, tag="nfs")
            nc.vector.tensor_copy(out=nfs[:], in_=ps3[:, :PC])
            for j in range(nw):
                o = (wg + j) * PC
                nc.sync.dma_start(out=nf_d[:, o : o + PC],
                                  in_=nfs[32 * j : 32 * j + 4, :])

        # ---------- edge stage (fp16) ----------
        u = big.tile([128, N_ndp], F16, tag="u")
        for wi in range(NWIN):
            o = wi * PC
            ps = psB.tile([128, PC], F32, tag="B")
            nc.tensor.matmul(ps[:, :PC], hw("we1h"), h3[:, o : o + PC],
                             start=True, stop=True)
            nc.scalar.activation(out=u[:, o : o + PC], in_=ps[:, :PC],
                                 func=AF.Identity, bias=b("be1h"), scale=1.0)

        z1 = big.tile([128, N_prp], F16, tag="z1")
        if N_prp > N_pr:
            nc.vector.memset(z1[:, N_pr:N_prp], 0.0)
        for ci, (n, gs, gk) in enumerate(chunks):
            no, po = node_off[ci], pair_off[ci]
            run_off = 0
            for i in range(n):
                rl = (n - i) * gk
                ui = u[:, no + i * gk : no + (i + 1) * gk]
                uib = ui.unsqueeze(1).broadcast_to([128, n - i, gk])
                uj = u[:, no + i * gk : no + n * gk].rearrange(
                    "p (m g) -> p m g", m=n - i)
                nc.vector.scalar_tensor_tensor(
                    out=z1[:, po + run_off : po + run_off + rl].rearrange(
                        "p (m g) -> p m g", m=n - i),
                    in0=uib, scalar=1.0, in1=uj, op0=ALU.mult, op1=ALU.add)
                run_off += rl

        # a1 = lrelu(z1) (bias be1 already inside via 2 x be1h)
        NPW = N_prp // PC
        a2f = big.tile([128, N_prp], F16, tag="a2f")
        for wi in range(NPW):
            o = wi * PC
            a1 = ptmp.tile([128, PC], F16, tag="a1")
            if wi % 2 == 0:
                nc.scalar.activation(out=a1[:], in_=z1[:, o : o + PC],
                                     func=AF.Prelu, scale=1.0, alpha=NEG)
            else:
                nc.vector.scalar_tensor_tensor(
                    out=a1[:], in0=z1[:, o : o + PC], scalar=NEG,
                    in1=z1[:, o : o + PC], op0=ALU.mult, op1=ALU.max)
            ps = psB.tile([128, PC], F32, tag="B")
            nc.tensor.matmul(ps[:, :PC], hw("we2"), a1[:],
                             start=True, stop=True)
            nc.scalar.activation(out=a2f[:, o : o + PC], in_=ps[:, :PC],
                                 func=AF.Prelu, bias=b("be2"), scale=1.0,
                                 alpha=NEG)
        # we3 col-tiled, 4 windows per psum tile
        for wg in range(0, NPW, 4):
            nw = min(4, NPW - wg)
            ps3 = psA.tile([128, 2 * PC], F32, tag="A")
            for j in range(nw):
                o = (wg + j) * PC
                nc.tensor.matmul(ps3[32 * j : 32 * j + 5, :PC], hw("we3"),
                                 a2f[:, o : o + PC], start=True, stop=True,
                                 tile_position=(0, 32 * j))
            eas = ptmp.tile([128, PC], F32, tag="eas")
            nc.vector.tensor_copy(out=eas[:], in_=ps3[:, :PC])
            for j in range(nw):
                o = (wg + j) * PC
                nc.sync.dma_start(out=ea_d[:, o : o + PC],
                                  in_=eas[32 * j : 32 * j + 5, :])


def gv64_slice(g64, go, gk):
    return g64[:, go : go + gk]


def _build_program(host):
    nc = bacc.Bacc("TRN2", debug=False)
    G_c, N_ndp, N_prp = host["G_c"], host["N_ndp"], host["N_prp"]
    gv_d = nc.dram_tensor("gv", [128, G_c], F32, kind="ExternalInput").ap()
    wf_d = nc.inline_tensor(host["WF"], name="wfpack").ap()
    wh_d = nc.inline_tensor(host["WH"], name="whpack").ap()
    bp_d = nc.inline_tensor(host["BP"], name="bpack").ap()
    pn_d = nc.dram_tensor("pn", [1, G_c], F32, kind="ExternalOutput").ap()
    nf_d = nc.dram_tensor("nf", [4, N_ndp], F32, kind="ExternalOutput").ap()
    ea_d = nc.dram_tensor("ea", [5, N_prp], F32, kind="ExternalOutput").ap()
    with tile.TileContext(nc) as tc:
        _emit(tc, nc, host, gv_d, pn_d, nf_d, ea_d, wf_d, wh_d, bp_d)
    nc.compile()
    return nc


_CACHE = {}


def _get_program(host):
    import hashlib
    hsh = hashlib.sha1(host["WF"].tobytes() + host["WH"].tobytes()
                       + host["BP"].tobytes()).hexdigest()
    key = (tuple(host["chunks"]), hsh, FAST, "v2")
    if key not in _CACHE:
        _CACHE[key] = _build_program(host)
    return _CACHE[key]


def _make_in_maps(host, gvec):
    chunks = host["chunks"]; goff = host["goff"]; G_c = host["G_c"]
    in_maps = []
    col_gid = np.full((NCORES, G_c), -1, np.int64)
    for c in range(NCORES):
        gvT = np.zeros((128, G_c), np.float32)
        for ci, (n, gs, gk) in enumerate(chunks):
            ids = host["core_ids_by_block"][n - 1][c][gs : gs + gk]
            go = goff[ci]
            col_gid[c, go : go + gk] = ids
            valid = ids >= 0
            if valid.any():
                gvT[:, go : go + gk][:, valid] = gvec[ids[valid]].T
        in_maps.append({"gv": gvT})
    return in_maps, col_gid


def _scatter_outputs(host, results, col_gid, B):
    chunks = host["chunks"]
    node_off = host["node_off"]; pair_off = host["pair_off"]; goff = host["goff"]
    N_ndp, N_prp = host["N_ndp"], host["N_prp"]
    nd_slot = np.full(N_ndp, -1, np.int64)
    nd_node = np.zeros(N_ndp, np.int64)
    pr_slot = np.full(N_prp, -1, np.int64)
    pr_i = np.zeros(N_prp, np.int64)
    pr_j = np.zeros(N_prp, np.int64)
    for ci, (n, gs, gk) in enumerate(chunks):
        no, po, go = node_off[ci], pair_off[ci], goff[ci]
        for i in range(n):
            sl = slice(no + i * gk, no + (i + 1) * gk)
            nd_slot[sl] = np.arange(go, go + gk)
            nd_node[sl] = i
        off = po
        for i in range(n):
            for j in range(i, n):
                sl = slice(off, off + gk)
                pr_slot[sl] = np.arange(go, go + gk)
                pr_i[sl] = i
                pr_j[sl] = j
                off += gk

    bn3, bf3, be3 = host["bn3"], host["bf3"], host["be3"]
    pred_num = np.zeros(B, np.float32)
    node_feats = np.zeros((B, MAXN, 4), np.float32)
    edge_attr = np.zeros((B, MAXN, MAXN, 5), np.float32)

    for c in range(NCORES):
        r = results[c]
        gid = col_gid[c]
        gvalid = gid >= 0
        pred_num[gid[gvalid]] = r["pn"][0, gvalid] + bn3[0]

        nd_gid = np.where(nd_slot >= 0, col_gid[c][np.maximum(nd_slot, 0)], -1)
        v = nd_gid >= 0
        node_feats[nd_gid[v], nd_node[v], :] = r["nf"][:, v].T + bf3[None, :]

        pr_gid = np.where(pr_slot >= 0, col_gid[c][np.maximum(pr_slot, 0)], -1)
        v = pr_gid >= 0
        vals = r["ea"][:, v].T + be3[None, :]
        edge_attr[pr_gid[v], pr_i[v], pr_j[v], :] = vals
        edge_attr[pr_gid[v], pr_j[v], pr_i[v], :] = vals

    return node_feats, edge_attr, pred_num


def kernel(global_vec, num_nodes, lap_table, params):
    gvec = np.asarray(global_vec, np.float32)
    nn_arr = np.asarray(num_nodes).astype(np.int64)
    B = gvec.shape[0]
    host = _prep_host(params, lap_table, nn_arr)
    nc = _get_program(host)
    in_maps, col_gid = _make_in_maps(host, gvec)
    res = run_bass_kernel_spmd(nc, in_maps, list(range(NCORES)))
    return _scatter_outputs(host, res.results, col_gid, B)
